# revision 10
# baseline (speedup 1.0000x reference)
"""Trainium2 Bass kernel for sliding-window multihead attention w/ (inverted) ALiBi.

Reference computation (B=4, S=2048, E=1024, H=16, D=64, W=512):
  proj = x @ w_in.T ; q,k,v = split(proj)          (per-head D=64)
  scores = (q @ k.T) * (1/8) + (q_idx - kv_idx) * slope_h     [ADDS bias]
  mask: 0 <= q_idx - kv_idx < W  (sliding causal window), block-0 pad masked
  out = softmax(scores) @ v ;  y = out @ w_out.T

Sharding: 8 cores = (batch b in 0..3) x (sequence half in 0..1).
Each core computes 1024 query tokens (2 blocks of W=512) for one batch.
KV context = 1536 tokens (prev block + own 2 blocks), zero-padded for the
first half. Host pre-transposes + bf16-casts the operands so every matmul
contraction dim lands on SBUF partitions with contiguous DMA.

Softmax exponent is assembled entirely inside the score matmul via four
augmented contraction rows (K = 64+4):
  khat rows: [kT | 1 | hi(w) | lo(w) | pmask(w)]
  qhat rows: [qT*SCALE | aug(t) | 1 | 1 | jbind(t)]
with hi+lo a double-bf16 split of s_h*(512-w) (per-key, abs err ~1e-2),
aug(t) = s_h*(t - min(g_q,511)) the per-query stability shift (its bf16
rounding is a per-query factor on e AND its denominator -> cancels exactly
in the softmax division), and pmask*jbind masking the nonexistent block
before sequence start. Window triangles are zeroed post-exp with grouped
affine_selects; denominators come from an appended ones-column in the PV
matmul lhsT (row 64 of oT = sum of e).
"""

import sys

sys.path.insert(0, "/opt/trn_rl_repo")

import numpy as np
import ml_dtypes

B, S, E, H, D = 4, 2048, 1024, 16, 64
W = 512
SCALE = 1.0 / np.sqrt(D)
TQ = 1024          # query tokens per core
TKV = 1536         # kv tokens per core (1 prev block + 2 own blocks)
NCORES = 8
NEG = -1.0e30

# k-tile -> (exp group, column offset in group); groups pack valid q-slices
_Q_LO = [max(0, t - 4) * 128 for t in range(8)]
_Q_HI = [min(4, t + 1) * 128 for t in range(8)]
_N_T = [hi - lo for lo, hi in zip(_Q_LO, _Q_HI)]
_GROUPS = [(0, 1), (2, 3), (4, 5), (6, 7)]
# column offsets bank-aligned: a matmul output may not cross a 512-col bank
_TMAP = {0: (0, 0), 1: (0, 128), 2: (1, 512), 3: (1, 0),
         4: (2, 0), 5: (2, 512), 6: (3, 0), 7: (3, 256)}
_GW = [384, 896, 896, 384]

_PROGRAM = None


def _build_program(repeat=0):
    import concourse.bass as bass
    import concourse.mybir as mybir
    import concourse.tile as tile
    from concourse import bacc
    import contextlib

    bf16 = mybir.dt.bfloat16
    f32 = mybir.dt.float32

    nc = bacc.Bacc("TRN2", target_bir_lowering=False, debug=False)

    xT_d = nc.dram_tensor("xT", [E, TKV], bf16, kind="ExternalInput").ap()
    w_inT_d = nc.dram_tensor("w_inT", [E, 3 * E], bf16, kind="ExternalInput").ap()
    w_outT_d = nc.dram_tensor("w_outT", [E, E], bf16, kind="ExternalInput").ap()
    qex_d = nc.dram_tensor("qex", [H, 4, TQ], bf16, kind="ExternalInput").ap()
    kex_d = nc.dram_tensor("kex", [H, 4, TKV], bf16, kind="ExternalInput").ap()
    y_d = nc.dram_tensor("y", [TQ, E], f32, kind="ExternalOutput").ap()

    ET = E // 128            # 8 e-tiles
    NKV = TKV // 128         # 12 kv t-tiles
    Exp = mybir.ActivationFunctionType.Exp

    with tile.TileContext(nc) as tc:
        with (
            tc.tile_pool(name="resident", bufs=1) as res,
            tc.tile_pool(name="wstream", bufs=2) as wst,
            tc.tile_pool(name="work", bufs=2) as work,
            tc.tile_pool(name="psS", bufs=2, space="PSUM") as psS,
            tc.tile_pool(name="psA", bufs=2, space="PSUM") as psA,
            tc.tile_pool(name="psO", bufs=2, space="PSUM") as psO,
        ):
          with (tc.For_i(0, repeat, 1) if repeat else contextlib.nullcontext()):
            # ---------------- resident loads ----------------
            xt = []
            for et in range(ET):
                t = res.tile([128, TKV], bf16, tag=f"xt{et}", name=f"xt{et}")
                nc.sync.dma_start(out=t, in_=xT_d[128 * et:128 * (et + 1), :])
                xt.append(t)
            woutT = []
            for et in range(ET):
                t = res.tile([128, E], bf16, tag=f"wo{et}", name=f"wo{et}")
                nc.sync.dma_start(out=t, in_=w_outT_d[128 * et:128 * (et + 1), :])
                woutT.append(t)

            # qhat[h]: [68, TQ] rows 0-63 = qT*SCALE, 64-67 = [aug,1,1,jbind]
            qhat = []
            for h in range(H):
                t = res.tile([68, TQ], bf16, tag=f"qh{h}", name=f"qh{h}")
                nc.sync.dma_start(out=t[64:68, :], in_=qex_d[h])
                qhat.append(t)
            # khat[h]: [68, TKV] rows 0-63 = kT, 64-67 = [1,hi,lo,pmask]
            khat = []
            for h in range(H):
                t = res.tile([68, TKV], bf16, tag=f"kh{h}", name=f"kh{h}")
                nc.sync.dma_start(out=t[64:68, :], in_=kex_d[h])
                khat.append(t)
            # vhat[tt]: [128, H*65]; per head 64 v cols + ones col
            vhat = []
            for tt in range(NKV):
                t = res.tile([128, H * 65], bf16, tag=f"vh{tt}", name=f"vh{tt}")
                nc.vector.memset(t, 1.0)     # ones cols survive; rest overwritten
                vhat.append(t)
            # o_normT[jb][et]: [128, W] bf16
            onrm = [[res.tile([128, W], bf16, tag=f"on{jb}_{et}", name=f"on{jb}_{et}")
                     for et in range(ET)] for jb in range(2)]

            # ---------------- emission helpers ----------------
            is_gt = mybir.AluOpType.is_gt
            is_ge = mybir.AluOpType.is_ge

            def emit_qk_chunk(jc):
                wt = []
                for et in range(ET):
                    t = wst.tile([128, 512], bf16, tag=f"wi{et}",
                                 name=f"wi{et}_{jc}")
                    nc.sync.dma_start(
                        out=t, in_=w_inT_d[128 * et:128 * (et + 1),
                                           512 * jc:512 * (jc + 1)])
                    wt.append(t)
                is_q = jc < 2
                tchunks = [(512, 1024), (1024, 1536)] if is_q else \
                          [(0, 512), (512, 1024), (1024, 1536)]
                for jt in range(4):
                    j0 = 512 * jc + 128 * jt
                    for (t0, t1) in tchunks:
                        ps = psA.tile([128, 512], f32, tag="mm", name="ps_qk")
                        for et in range(ET):
                            nc.tensor.matmul(
                                ps[:, :t1 - t0],
                                wt[et][:, 128 * jt:128 * (jt + 1)],
                                xt[et][:, t0:t1],
                                start=(et == 0), stop=(et == ET - 1))
                        for sub in range(2):
                            if is_q:
                                h = j0 // 64 + sub
                                nc.scalar.mul(
                                    qhat[h][0:64, t0 - 512:t1 - 512],
                                    ps[64 * sub:64 * sub + 64, :t1 - t0], SCALE)
                            else:
                                h = (j0 - 1024) // 64 + sub
                                nc.vector.tensor_copy(
                                    khat[h][0:64, t0:t1],
                                    ps[64 * sub:64 * sub + 64, :t1 - t0])

            def emit_v_chunk(jc):
                h0 = (jc - 4) * 8
                wt = []
                for et in range(ET):
                    t = wst.tile([128, 512], bf16, tag=f"wi{et}",
                                 name=f"wv{et}_{jc}")
                    nc.sync.dma_start(
                        out=t, in_=w_inT_d[128 * et:128 * (et + 1),
                                           512 * jc:512 * (jc + 1)])
                    wt.append(t)
                for tt in range(NKV):
                    ps = psA.tile([128, 512], f32, tag="mm", name="ps_v")
                    for et in range(ET):
                        nc.tensor.matmul(
                            ps, xt[et][:, 128 * tt:128 * (tt + 1)], wt[et],
                            start=(et == 0), stop=(et == ET - 1))
                    # one strided copy: psum [128,(8,64)] -> vhat 65-strided
                    vbase = vhat[tt][:, 65 * h0:65 * h0 + 64]
                    vdst = bass.AP(tensor=vbase.tensor, offset=vbase.offset,
                                   ap=[vbase.ap[0], [65, 8], [1, 64]])
                    psrc = bass.AP(tensor=ps.tensor, offset=ps.offset,
                                   ap=[ps.ap[0], [64, 8], [1, 64]])
                    nc.vector.tensor_copy(vdst, psrc)

            def emit_attn(h):
                for jb in range(2):
                    oT = psO.tile([65, W], f32, tag="ot", name=f"ot{jb}_{h}")
                    esb = []
                    for g, ts in enumerate(_GROUPS):
                        sc = psS.tile([128, 896], f32, tag="sc",
                                      name=f"sc{jb}_{h}_{g}")
                        for t in ts:
                            _, coff = _TMAP[t]
                            w0 = jb * W + 128 * t
                            nc.tensor.matmul(
                                sc[:, coff:coff + _N_T[t]],
                                khat[h][:, w0:w0 + 128],
                                qhat[h][:, jb * W + _Q_LO[t]:jb * W + _Q_HI[t]],
                                start=True, stop=True)
                        eg = work.tile([128, 896], bf16, tag="et", bufs=6,
                                       name=f"et{jb}_{h}_{g}")
                        nc.scalar.activation(eg[:, :_GW[g]], sc[:, :_GW[g]], Exp)
                        esb.append(eg)
                    # grouped triangle masks
                    # delta=0 (keep p>f): g0 zones @0,256 ; g1 zones @384,768
                    for g, base_off, stride in ((0, 0, 256), (1, 384, 384)):
                        bap = esb[g][:, base_off:base_off + stride + 128]
                        zap = bass.AP(tensor=bap.tensor, offset=bap.offset,
                                      ap=[bap.ap[0], [stride, 2], [1, 128]])
                        nc.gpsimd.affine_select(
                            zap, zap, pattern=[[0, 2], [-1, 128]],
                            compare_op=is_gt, fill=0.0, base=0,
                            channel_multiplier=1)
                    # delta=4 (keep p<=f): g2 zones @0,512 ; g3 zones @0,256
                    for g, base_off, stride in ((2, 0, 512), (3, 0, 256)):
                        bap = esb[g][:, base_off:base_off + stride + 128]
                        zap = bass.AP(tensor=bap.tensor, offset=bap.offset,
                                      ap=[bap.ap[0], [stride, 2], [1, 128]])
                        nc.gpsimd.affine_select(
                            zap, zap, pattern=[[0, 2], [1, 128]],
                            compare_op=is_ge, fill=0.0, base=0,
                            channel_multiplier=-1)
                    # PV: oT[:, qs-slice] = sum_t vhat^T @ eT
                    for qs in range(4):
                        for dlt in range(5):
                            t = qs + dlt
                            g, coff = _TMAP[t]
                            kvt = jb * 4 + t
                            c0 = coff + 128 * qs - _Q_LO[t]
                            nc.tensor.matmul(
                                oT[:, 128 * qs:128 * (qs + 1)],
                                vhat[kvt][:, 65 * h:65 * h + 65],
                                esb[g][:, c0:c0 + 128],
                                start=(dlt == 0), stop=(dlt == 4))
                    # normalize: o_norm = o_un * (1/denom), denom = row 64
                    rec = work.tile([1, W], f32, tag="rec", name=f"rec{jb}_{h}")
                    nc.vector.reciprocal(rec, oT[64:65, :])
                    rb = work.tile([64, W], f32, tag="rb", name=f"rb{jb}_{h}")
                    nc.gpsimd.partition_broadcast(rb, rec)
                    nc.vector.tensor_mul(
                        onrm[jb][h // 2][64 * (h % 2):64 * (h % 2) + 64, :],
                        oT[0:64, :], rb)

            def emit_outproj(jb):
                for tt4 in range(4):
                    ysb = work.tile([128, E], f32, tag="ysb", name=f"y{jb}_{tt4}")
                    for jc in range(2):
                        ps = psA.tile([128, 512], f32, tag="mm", name="ps_y")
                        for et in range(ET):
                            nc.tensor.matmul(
                                ps,
                                onrm[jb][et][:, 128 * tt4:128 * (tt4 + 1)],
                                woutT[et][:, 512 * jc:512 * (jc + 1)],
                                start=(et == 0), stop=(et == ET - 1))
                        nc.scalar.copy(ysb[:, 512 * jc:512 * (jc + 1)], ps)
                    r0 = jb * W + 128 * tt4
                    nc.sync.dma_start(out=y_d[r0:r0 + 128, :], in_=ysb)

            # ------------- emission schedule: overlap attention ACT/Pool -------------
            # with the back half of the in-projection PE stream.
            emit_qk_chunk(0)     # q heads 0-7
            emit_qk_chunk(2)     # k heads 0-7
            emit_v_chunk(4)      # v heads 0-7
            emit_attn(0); emit_attn(1)
            emit_qk_chunk(1)     # q heads 8-15
            emit_attn(2); emit_attn(3); emit_attn(4)
            emit_qk_chunk(3)     # k heads 8-15
            emit_attn(5); emit_attn(6); emit_attn(7)
            emit_v_chunk(5)      # v heads 8-15
            for h in range(8, 16):
                emit_attn(h)
            emit_outproj(0)
            emit_outproj(1)

    nc.compile()
    return nc


def _host_inputs(x, w_in, w_out):
    """Build the 8 per-core input maps (host-side shard/transpose/cast)."""
    bf = ml_dtypes.bfloat16
    w_inT = np.ascontiguousarray(w_in.astype(np.float32).T).astype(bf)
    w_outT = np.ascontiguousarray(w_out.astype(np.float32).T).astype(bf)

    slopes = np.exp2(-(np.arange(1, H + 1, dtype=np.float64) * 8.0 / H))
    t = np.arange(TQ, dtype=np.float64)
    w = np.arange(TKV, dtype=np.float64)

    # per-key double-bf16 rows: hi+lo ~= s_h*(512-w)
    bk = slopes[:, None] * (W - w)[None, :]          # [H, TKV]
    hi = bk.astype(bf)
    lo = (bk - hi.astype(np.float64)).astype(bf)
    ones_kv = np.ones((TKV,), bf)

    jbind = (t < W).astype(bf)                       # [TQ]
    ones_q = np.ones((TQ,), bf)

    in_maps = []
    for c in range(NCORES):
        b, half = c // 2, c % 2
        if half == 0:
            xkv = np.concatenate(
                [np.zeros((W, E), np.float32), np.asarray(x[b, 0:TQ], np.float32)], 0)
        else:
            xkv = np.asarray(x[b, S - TKV:S], np.float32)
        xT = np.ascontiguousarray(xkv.T).astype(bf)

        g_q = half * TQ + t
        aug = slopes[:, None] * (t[None, :] - np.minimum(g_q, W - 1.0)[None, :])
        qex = np.empty((H, 4, TQ), bf)
        qex[:, 0, :] = aug.astype(bf)
        qex[:, 1, :] = ones_q
        qex[:, 2, :] = ones_q
        qex[:, 3, :] = jbind

        pmask = np.zeros((TKV,), np.float64)
        if half == 0:
            pmask[0:W] = NEG
        kex = np.empty((H, 4, TKV), bf)
        kex[:, 0, :] = ones_kv
        kex[:, 1, :] = hi
        kex[:, 2, :] = lo
        kex[:, 3, :] = pmask.astype(bf)

        in_maps.append({
            "xT": xT, "w_inT": w_inT, "w_outT": w_outT,
            "qex": qex, "kex": kex,
        })
    return in_maps


_RUNNER = None


def _get_runner():
    """Build (once) a cached jax-jitted SPMD executor for the bass program,
    mirroring concourse.bass2jax.run_bass_via_pjrt's multi-core path."""
    global _PROGRAM, _RUNNER
    if _RUNNER is not None:
        return _RUNNER
    if _PROGRAM is None:
        _PROGRAM = _build_program()
    nc = _PROGRAM

    import jax
    from jax.sharding import Mesh, PartitionSpec
    from jax.experimental.shard_map import shard_map
    import concourse.mybir as mybir
    from concourse import bass2jax

    bass2jax.install_neuronx_cc_hook()

    partition_name = nc.partition_id_tensor.name if nc.partition_id_tensor else None
    in_names, out_names, out_avals, zero_outs = [], [], [], []
    for alloc in nc.m.functions[0].allocations:
        if not isinstance(alloc, mybir.MemoryLocationSet):
            continue
        name = alloc.memorylocations[0].name
        if alloc.kind == "ExternalInput":
            if name != partition_name:
                in_names.append(name)
        elif alloc.kind == "ExternalOutput":
            out_names.append(name)
            shape = tuple(alloc.tensor_shape)
            dtype = mybir.dt.np(alloc.dtype)
            out_avals.append(jax.core.ShapedArray(shape, dtype))
            zero_outs.append(np.zeros(shape, dtype))
    n_params = len(in_names)
    n_outs = len(out_avals)
    all_in_names = list(in_names) + list(out_names)
    if partition_name is not None:
        all_in_names.append(partition_name)
    donate = tuple(range(n_params, n_params + n_outs))

    def _body(*args):
        operands = list(args)
        if partition_name is not None:
            operands.append(bass2jax.partition_id_tensor())
        outs = bass2jax._bass_exec_p.bind(
            *operands,
            out_avals=tuple(out_avals),
            in_names=tuple(all_in_names),
            out_names=tuple(out_names),
            lowering_input_output_aliases=(),
            sim_require_finite=True,
            sim_require_nnan=True,
            nc=nc,
        )
        return tuple(outs)

    devices = jax.devices()[:NCORES]
    mesh = Mesh(np.asarray(devices), ("core",))
    in_specs = (PartitionSpec("core"),) * (n_params + n_outs)
    out_specs = (PartitionSpec("core"),) * n_outs
    sharded = jax.jit(
        shard_map(_body, mesh=mesh, in_specs=in_specs, out_specs=out_specs,
                  check_rep=False),
        donate_argnums=donate, keep_unused=True)

    _RUNNER = {
        "fn": sharded, "in_names": in_names, "out_names": out_names,
        "zero_outs": zero_outs, "out_avals": out_avals,
    }
    return _RUNNER


def _run_spmd(in_maps):
    r = _get_runner()
    concat_in = [
        np.concatenate([m[name] for m in in_maps], axis=0) for name in r["in_names"]
    ]
    concat_zeros = [
        np.zeros((NCORES * z.shape[0], *z.shape[1:]), z.dtype) for z in r["zero_outs"]
    ]
    out_arrs = r["fn"](*concat_in, *concat_zeros)
    return out_arrs


def kernel(x, w_in, w_out):
    in_maps = _host_inputs(x, w_in, w_out)
    out_arrs = _run_spmd(in_maps)
    r = _RUNNER
    yi = r["out_names"].index("y")
    yall = np.asarray(out_arrs[yi]).reshape(NCORES, TQ, E)
    y = np.empty((B, S, E), np.float32)
    for c in range(NCORES):
        b, half = c // 2, c % 2
        y[b, half * TQ:(half + 1) * TQ, :] = yall[c]
    return y
